# revision 1
# baseline (speedup 1.0000x reference)
"""ChebNet (K=2) GNN message passing on 8 TRN2 NeuronCores.

Strategy (edge sharding by destination stripe):
  - Sort edges by destination node; core c owns destinations
    [c*6272, (c+1)*6272) (N padded 50000 -> 50176 = 8*6272).
  - Host passes only integer index/offset data: per-edge source (col)
    indices, per-edge local destination slots, and the CSR row_ptr for
    the stripe. All floating point math (degree -> rsqrt, feature
    scaling, the two sparse matvecs, dense matmuls, log_softmax) runs
    on device.
  - L_hat matvec is factored: (L x)[i] = -dinv[i] * sum_e dinv[col_e] x[col_e]
    so the gather table is x' = dinv (.) x, built on device, AllGathered
    across the 8 cores (1.6MB/rank), then gathered per-edge with
    nc.gpsimd.dma_gather (InstDMAGatherAnt, 256B rows, int16 indices ->
    table split in two 25088-row halves; <=512 indices per gather, which
    larger single-packet gathers wedge the device). Per 128-edge block
    a one-hot matrix S
    (iota == localdest) is built on DVE and PE computes G.T @ S into a
    PSUM window [64 feats, 128 dests], accumulated over the window's
    blocks; the PSUM drain fuses the -dinv[dest] scale.
  - Layer 2 repeats with h' = dinv (.) relu(...) via a second AllGather.
"""

import os
import sys

import numpy as np

sys.path.insert(0, "/opt/trn_rl_repo")

import concourse.bacc as bacc
import concourse.bass as bass
import concourse.tile as tile
from concourse import mybir
from concourse.masks import make_identity

FP32 = mybir.dt.float32
BF16 = mybir.dt.bfloat16
I32 = mybir.dt.int32
TDT = BF16 if os.environ.get("CHEB_TABLE_DT", "fp32") == "bf16" else FP32
TSZ = 2 if TDT is BF16 else 4
# One dma_gather per window-half (multi-packet). Single-packet gathers
# above 512 indices wedge the device; multi-packet is stable at 1280.
GMAX = int(os.environ.get("CHEB_GMAX", "10"))        # blocks per dma_gather
SINGLE_PACKET = os.environ.get("CHEB_SP", "0") == "1"

N = 50000
E = 800000
F = 64          # in dim
HID = 64
OUT = 40
C = 8           # cores
SN = 6272       # nodes per stripe (49 * 128)
NP = SN * C     # padded node count 50176
W = SN // 128   # 49 windows per core
HALF = NP // 2  # 25088, int16-indexable table half
AX = mybir.AxisListType


# ---------------------------------------------------------------- host side


def _pack(edge_index: np.ndarray):
    """Integer-only preprocessing: sort/partition/pad the edge list.

    dma_gather uses int16 indices, so the node table is split into two
    halves of HALF=25088 rows; each window's edges are grouped by
    source half (lo first, then hi), each group padded to whole
    128-edge blocks. Block counts are maxed over cores so the SPMD
    program is uniform.

    Returns:
      idx16  [C, 128, NBtot*8] int16 - wrapped (16-partition) gather
             indices, replicated down partition groups.
      ldest  [C, 128, NBtot] f32 - local dest slot (0..127) or 255 pad.
      rp_a/rp_b [C, 128, W] f32 - CSR row_ptr (degree = rp_b - rp_a).
      groups: list over windows of (lo_block_base, lo_nblocks,
              hi_block_base, hi_nblocks).
    """
    row = np.asarray(edge_index[0], dtype=np.int64)
    col = np.asarray(edge_index[1], dtype=np.int64)

    cnt = np.bincount(row, minlength=NP)
    rp = np.zeros(NP + 1, dtype=np.int64)
    np.cumsum(cnt, out=rp[1:])

    order = np.argsort(row, kind="stable")
    rs = row[order]
    cs = col[order]
    keep = rs != cs
    rs = rs[keep]
    cs = cs[keep]

    # group id: (core, window, half) -> counts
    win = rs >> 7
    half = (cs >= HALF).astype(np.int64)
    gid = win * 2 + half
    # sort by gid (stable keeps dest order within groups; order within a
    # group is irrelevant anyway)
    gorder = np.argsort(gid, kind="stable")
    rs = rs[gorder]
    cs = cs[gorder]
    gid = gid[gorder]

    gcnt = np.bincount(gid, minlength=C * W * 2).reshape(C, W, 2)
    nbw = (gcnt.max(axis=0) + 127) // 128          # [W, 2]
    # guarantee each window has at least one block so PSUM is initialized
    empty = nbw.sum(axis=1) == 0
    nbw[empty, 0] = 1
    nbtot = int(nbw.sum())

    # block base per (w, half): lo blocks then hi blocks within a window
    wb = np.zeros(W * 2 + 1, dtype=np.int64)
    np.cumsum(nbw.reshape(-1), out=wb[1:])
    groups = [
        (int(wb[2 * w]), int(nbw[w, 0]), int(wb[2 * w + 1]), int(nbw[w, 1]))
        for w in range(W)
    ]

    flat_idx = np.zeros((C, nbtot * 128), dtype=np.int16)
    ldest = np.full((C, 128, nbtot), 255.0, dtype=np.float32)

    starts = np.zeros(C * W * 2 + 1, dtype=np.int64)
    np.cumsum(gcnt.reshape(-1), out=starts[1:])
    for c in range(C):
        for w in range(W):
            for h in range(2):
                g = (c * W + w) * 2 + h
                s, e = starts[g], starts[g + 1]
                m = e - s
                if m == 0:
                    continue
                base = wb[2 * w + h]
                ei = np.arange(m)
                b = base + (ei >> 7)
                p = ei & 127
                flat_idx[c, b * 128 + p] = (cs[s:e] - h * HALF).astype(np.int16)
                ldest[c, p, b] = (rs[s:e] - (c * SN + w * 128)).astype(
                    np.float32)

    # wrap: index i at [i % 16, i // 16], replicated down 8 partition groups
    S_tot = nbtot * 8
    idx16 = np.zeros((C, 128, S_tot), dtype=np.int16)
    wrapped = flat_idx.reshape(C, S_tot, 16).transpose(0, 2, 1)  # [C,16,S]
    for grp in range(8):
        idx16[:, grp * 16:(grp + 1) * 16, :] = wrapped

    rpf = rp.astype(np.float64)
    assert rpf.max() < 2 ** 24
    idx = (np.arange(W)[None, :] * 128 + np.arange(128)[:, None])
    rp_a = np.zeros((C, 128, W), dtype=np.float32)
    rp_b = np.zeros((C, 128, W), dtype=np.float32)
    for c in range(C):
        base = c * SN
        rp_a[c] = rpf[base + idx].astype(np.float32)
        rp_b[c] = rpf[base + idx + 1].astype(np.float32)

    return idx16, ldest, rp_a, rp_b, groups, nbtot


# -------------------------------------------------------------- bass program


def _build(groups, nbtot, replay=0):
    nc = bacc.Bacc(
        "TRN2",
        target_bir_lowering=False,
        debug=False,
        num_devices=C,
    )

    # --- I/O
    x_st = nc.dram_tensor("x_st", [SN, F], FP32, kind="ExternalInput").ap()
    idx16 = nc.dram_tensor("idx16", [128, nbtot * 8],
                           mybir.dt.int16, kind="ExternalInput").ap()
    ldst = nc.dram_tensor("ldst", [128, nbtot], FP32, kind="ExternalInput").ap()
    rpa = nc.dram_tensor("rpa", [128, W], FP32, kind="ExternalInput").ap()
    rpb = nc.dram_tensor("rpb", [128, W], FP32, kind="ExternalInput").ap()
    w01 = nc.dram_tensor("w01", [F, HID], FP32, kind="ExternalInput").ap()
    w11 = nc.dram_tensor("w11", [F, HID], FP32, kind="ExternalInput").ap()
    b1 = nc.dram_tensor("b1", [HID, 1], FP32, kind="ExternalInput").ap()
    w02 = nc.dram_tensor("w02", [HID, OUT], FP32, kind="ExternalInput").ap()
    w12 = nc.dram_tensor("w12", [HID, OUT], FP32, kind="ExternalInput").ap()
    b2 = nc.dram_tensor("b2", [OUT, 1], FP32, kind="ExternalInput").ap()
    out = nc.dram_tensor("out", [SN, OUT], FP32, kind="ExternalOutput").ap()

    # --- internal DRAM (collective bounce + gather tables)
    xp_b = nc.dram_tensor("xp_b", [SN, F], TDT).ap()
    xp_full = nc.dram_tensor("xp_full", [NP, F], TDT, addr_space="Shared").ap()
    hp_b = nc.dram_tensor("hp_b", [SN, F], TDT).ap()
    hp_full = nc.dram_tensor("hp_full", [NP, F], TDT, addr_space="Shared").ap()
    dinv_d = nc.dram_tensor("dinv_d", [W, 128], FP32).ap()

    nbmax = max(max(g[1], g[3]) for g in groups)
    rg = [list(range(C))]

    with tile.TileContext(nc) as tc:
        with (
            tc.tile_pool(name="const", bufs=1) as cpool,
            tc.tile_pool(name="big", bufs=1) as bpool,
            tc.tile_pool(name="work", bufs=(6 if GMAX <= 4 else 3)) as wpool,
            tc.tile_pool(name="spool", bufs=2) as spool,
            tc.tile_pool(name="psw", bufs=4, space="PSUM") as psw,
            tc.tile_pool(name="psd", bufs=2, space="PSUM") as psd,
            tc.tile_pool(name="pst", bufs=2, space="PSUM") as pst,
        ):
            # ---- constants
            ident = cpool.tile([128, 128], FP32, tag="ident")
            make_identity(nc, ident[:])
            iota_i = cpool.tile([128, 128], I32, tag="iota_i")
            nc.gpsimd.iota(iota_i[:], pattern=[[1, 128]], base=0,
                           channel_multiplier=0)
            iota_f = cpool.tile([128, 128], TDT, tag="iota_f")
            nc.vector.tensor_copy(iota_f[:], iota_i[:])

            w01_s = cpool.tile([F, HID], FP32, tag="w01")
            w11_s = cpool.tile([F, HID], FP32, tag="w11")
            w02_s = cpool.tile([HID, OUT], FP32, tag="w02")
            w12_s = cpool.tile([HID, OUT], FP32, tag="w12")
            b1_s = cpool.tile([HID, 1], FP32, tag="b1")
            b2_s = cpool.tile([OUT, 1], FP32, tag="b2")
            for dst, src in ((w01_s, w01), (w11_s, w11), (w02_s, w02),
                             (w12_s, w12), (b1_s, b1), (b2_s, b2)):
                nc.sync.dma_start(out=dst[:], in_=src)

            idx_s = cpool.tile([128, nbtot * 8], mybir.dt.int16, tag="idx16")
            nc.sync.dma_start(out=idx_s[:], in_=idx16)
            ldst_f = cpool.tile([128, nbtot], FP32, tag="ldst_f")
            nc.sync.dma_start(out=ldst_f[:], in_=ldst)
            if TDT is FP32:
                ldst_s = ldst_f
            else:
                ldst_s = cpool.tile([128, nbtot], TDT, tag="ldst")
                nc.vector.tensor_copy(ldst_s[:], ldst_f[:])

            # ---- degree -> dinv  [128, W] (node n = w*128 + p)
            rpa_s = cpool.tile([128, W], FP32, tag="rpa")
            rpb_s = cpool.tile([128, W], FP32, tag="rpb")
            nc.sync.dma_start(out=rpa_s[:], in_=rpa)
            nc.sync.dma_start(out=rpb_s[:], in_=rpb)
            deg = cpool.tile([128, W], FP32, tag="deg")
            nc.vector.tensor_tensor(out=deg[:], in0=rpb_s[:], in1=rpa_s[:],
                                    op=mybir.AluOpType.subtract)
            degc = cpool.tile([128, W], FP32, tag="degc")
            nc.vector.tensor_scalar_max(degc[:], deg[:], 1.0)
            rec = cpool.tile([128, W], FP32, tag="rec")
            nc.vector.reciprocal(rec[:], degc[:])
            rsq = cpool.tile([128, W], FP32, tag="rsq")
            nc.scalar.sqrt(rsq[:], rec[:])
            msk = cpool.tile([128, W], FP32, tag="msk")
            nc.vector.tensor_scalar(out=msk[:], in0=deg[:], scalar1=0.5,
                                    scalar2=None, op0=mybir.AluOpType.is_ge)
            dinv = cpool.tile([128, W], FP32, tag="dinv")
            nc.vector.tensor_tensor(out=dinv[:], in0=rsq[:], in1=msk[:],
                                    op=mybir.AluOpType.mult)

            # negative dinv replicated over 64 partitions, free-dim-major:
            # transpose [128, W] -> [W, 128], negate, bounce via DRAM with a
            # partition-broadcast reload.
            ps_dt = pst.tile([W, 128], FP32, tag="pt")
            nc.tensor.transpose(out=ps_dt[:], in_=dinv[:], identity=ident[:])
            dt_s = cpool.tile([W, 128], FP32, tag="dt")
            nc.vector.tensor_scalar_mul(dt_s[:], ps_dt[:], -1.0)
            nc.sync.dma_start(out=dinv_d, in_=dt_s[:])
            ndrep = bpool.tile([F, SN], FP32, tag="ndrep")
            dflat = dinv_d.rearrange("w p -> (w p)")
            nc.sync.dma_start(
                out=ndrep[:],
                in_=bass.AP(dflat.tensor, dflat.offset, [[0, F], [1, SN]]),
            )

            # ---- x stripe load, x' = dinv (.) x, xT
            x_sb = bpool.tile([128, W * F], FP32, tag="nm_a")
            nc.sync.dma_start(
                out=x_sb[:].rearrange("p (w f) -> p w f", w=W),
                in_=x_st.rearrange("(w p) f -> p w f", p=128))
            xp_sb = bpool.tile([128, W * F], TDT, tag="nm_b")
            for w in range(W):
                nc.vector.tensor_scalar_mul(
                    xp_sb[:, w * F:(w + 1) * F],
                    x_sb[:, w * F:(w + 1) * F],
                    dinv[:, w:w + 1],
                )
            nc.sync.dma_start(
                out=xp_b.rearrange("(w p) f -> p w f", p=128),
                in_=xp_sb[:].rearrange("p (w f) -> p w f", w=W))
            nc.gpsimd.collective_compute(
                "AllGather",
                mybir.AluOpType.bypass,
                ins=[xp_b],
                outs=[xp_full],
                replica_groups=rg,
            )

            xT = bpool.tile([F, SN], FP32, tag="xT_tx2T")
            for w in range(W):
                ps_t = pst.tile([F, 128], FP32, tag="pt")
                nc.tensor.transpose(out=ps_t[:], in_=x_sb[:, w * F:(w + 1) * F],
                                    identity=ident[:])
                nc.vector.tensor_copy(xT[:, w * 128:(w + 1) * 128], ps_t[:])

            # ---- sparse matvec: table [NP, F] -> acc.T [F, SN], scaled -dinv
            g0 = wpool.tile([128, min(nbmax, GMAX) * F], TDT, tag="G0")
            nc.vector.memset(g0[:], 0.0)

            nwmax = max(g[1] + g[3] for g in groups)

            def matvec(table_ap, dst, do_gather=True, do_compute=True):
                for w in range(W):
                    lo_b, lo_n, hi_b, hi_n = groups[w]
                    pw = psw.tile([F, 128], FP32, tag="pw", name="pw") if do_compute \
                        else None
                    ntot = lo_n + hi_n
                    if do_compute:
                        # one one-hot build per window (lo+hi blocks are
                        # contiguous starting at lo_b)
                        s = spool.tile([128, nwmax * 128], TDT, tag="S",
                                       name="s")
                        io_ap = iota_f[:]
                        ld_ap = ldst_s[:, lo_b:lo_b + ntot]
                        nc.vector.tensor_tensor(
                            out=s[:, : ntot * 128].rearrange(
                                "p (b q) -> p b q", b=ntot),
                            in0=bass.AP(io_ap.tensor, io_ap.offset,
                                        [io_ap.ap[0], [0, ntot],
                                         io_ap.ap[1]]),
                            in1=bass.AP(ld_ap.tensor, ld_ap.offset,
                                        [ld_ap.ap[0], ld_ap.ap[1],
                                         [0, 128]]),
                            op=mybir.AluOpType.is_equal)
                    done = 0
                    for base0, n0, hoff in ((lo_b, lo_n, 0),
                                            (hi_b, hi_n, HALF)):
                        for sub in range(0, n0, GMAX):
                            base = base0 + sub
                            n = min(GMAX, n0 - sub)
                            if n == 0:
                                continue
                            if do_gather:
                                g = wpool.tile(
                                    [128, min(nbmax, GMAX) * F], TDT, tag="G")
                                nc.gpsimd.dma_gather(
                                    out_ap=g[:, : n * F].rearrange(
                                        "p (b f) -> p b f", b=n),
                                    in_ap=table_ap[hoff:hoff + HALF, :],
                                    idxs_ap=idx_s[:, base * 8:(base + n) * 8],
                                    num_idxs=n * 128,
                                    num_idxs_reg=n * 128,
                                    elem_size=F,
                                    single_packet=SINGLE_PACKET,
                                )
                            else:
                                g = g0
                            if not do_compute:
                                nc.vector.tensor_copy(
                                    dst[:1, w * 128:w * 128 + 1], g[:1, :1])
                                done += n
                                continue
                            for b in range(n):
                                sb = base - lo_b + b
                                nc.tensor.matmul(
                                    out=pw[:], lhsT=g[:, b * F:(b + 1) * F],
                                    rhs=s[:, sb * 128:(sb + 1) * 128],
                                    start=(done == 0),
                                    stop=(done == ntot - 1))
                                done += 1
                    if do_compute:
                        nc.vector.tensor_tensor(
                            out=dst[:, w * 128:(w + 1) * 128], in0=pw[:],
                            in1=ndrep[:, w * 128:(w + 1) * 128],
                            op=mybir.AluOpType.mult)

            txT = bpool.tile([F, SN], FP32, tag="txT")
            hT = bpool.tile([HID, SN], FP32, tag="hT")
            hp_sb = bpool.tile([128, W * F], TDT, tag="nm_a")
            tx2T = bpool.tile([F, SN], FP32, tag="xT_tx2T")
            out_sb = bpool.tile([128, W * OUT], FP32, tag="nm_b")
            nchunk = (SN + 511) // 512

            # ---- dense layer 1: hT = relu(W01.T @ xT + W11.T @ txT + b1)
            def dense1():
                for i in range(nchunk):
                    lo = i * 512
                    m = min(512, SN - lo)
                    pd = psd.tile([HID, 512], FP32, tag="pd")
                    nc.tensor.matmul(out=pd[:, :m], lhsT=w01_s[:],
                                     rhs=xT[:, lo:lo + m], start=True,
                                     stop=False)
                    nc.tensor.matmul(out=pd[:, :m], lhsT=w11_s[:],
                                     rhs=txT[:, lo:lo + m], start=False,
                                     stop=True)
                    nc.scalar.activation(
                        out=hT[:, lo:lo + m], in_=pd[:, :m],
                        func=mybir.ActivationFunctionType.Relu,
                        bias=b1_s[:], scale=1.0)

            # ---- h' = dinv (.) h  (node-major)
            def hp_compute():
                for w in range(W):
                    ps_h = pst.tile([128, F], FP32, tag="pt")
                    nc.tensor.transpose(out=ps_h[:],
                                        in_=hT[:, w * 128:(w + 1) * 128],
                                        identity=ident[:F, :F])
                    nc.vector.tensor_scalar_mul(
                        hp_sb[:, w * F:(w + 1) * F], ps_h[:], dinv[:, w:w + 1])

            matvec(xp_full, txT)
            dense1()
            hp_compute()
            nc.sync.dma_start(
                out=hp_b.rearrange("(w p) f -> p w f", p=128),
                in_=hp_sb[:].rearrange("p (w f) -> p w f", w=W))
            nc.gpsimd.collective_compute(
                "AllGather",
                mybir.AluOpType.bypass,
                ins=[hp_b],
                outs=[hp_full],
                replica_groups=rg,
            )

            # ---- sparse matvec 2
            matvec(hp_full, tx2T)

            # ---- dense layer 2 + bias + transpose + log_softmax
            # out = o - ln(sum(exp(o))); chunked so oT never materializes
            def dense2_softmax():
                o_all = bpool.tile([128, W * OUT], FP32, tag="o_all")
                for i in range(nchunk):
                    lo = i * 512
                    m = min(512, SN - lo)
                    pd = psd.tile([OUT, 512], FP32, tag="pd")
                    nc.tensor.matmul(out=pd[:, :m], lhsT=w02_s[:],
                                     rhs=hT[:, lo:lo + m], start=True,
                                     stop=False)
                    nc.tensor.matmul(out=pd[:, :m], lhsT=w12_s[:],
                                     rhs=tx2T[:, lo:lo + m], start=False,
                                     stop=True)
                    ob = wpool.tile([OUT, 512], FP32, tag="ob")
                    nc.vector.tensor_scalar(
                        out=ob[:, :m], in0=pd[:, :m], scalar1=b2_s[:],
                        scalar2=None, op0=mybir.AluOpType.add)
                    for j in range(m // 128):
                        w = i * 4 + j
                        ps_o = pst.tile([128, OUT], FP32, tag="pt")
                        nc.tensor.transpose(
                            out=ps_o[:], in_=ob[:, j * 128:(j + 1) * 128],
                            identity=ident[:OUT, :OUT])
                        nc.vector.tensor_copy(
                            o_all[:, w * OUT:(w + 1) * OUT], ps_o[:])
                e_all = bpool.tile([128, W * OUT], FP32, tag="e_all")
                nc.scalar.activation(out=e_all[:], in_=o_all[:],
                                     func=mybir.ActivationFunctionType.Exp)
                ssum = wpool.tile([128, W], FP32, tag="ssum")
                nc.vector.tensor_reduce(
                    out=ssum[:],
                    in_=e_all[:].rearrange("p (w q) -> p w q", w=W),
                    axis=AX.X, op=mybir.AluOpType.add)
                lns = wpool.tile([128, W], FP32, tag="lns")
                nc.scalar.activation(out=lns[:], in_=ssum[:],
                                     func=mybir.ActivationFunctionType.Ln)
                ln_ap = lns[:]
                nc.vector.tensor_tensor(
                    out=out_sb[:].rearrange("p (w q) -> p w q", w=W),
                    in0=o_all[:].rearrange("p (w q) -> p w q", w=W),
                    in1=bass.AP(ln_ap.tensor, ln_ap.offset,
                                [ln_ap.ap[0], ln_ap.ap[1], [0, OUT]]),
                    op=mybir.AluOpType.subtract)

            dense2_softmax()
            nc.sync.dma_start(
                out=out.rearrange("(w p) f -> p w f", p=128),
                in_=out_sb[:].rearrange("p (w f) -> p w f", w=W))

            rm = os.environ.get("CHEB_RM", "full")
            for _ in range(replay):
                matvec(xp_full, txT,
                       do_gather=(rm != "nogather"),
                       do_compute=(rm != "gatheronly"))
                matvec(hp_full, tx2T,
                       do_gather=(rm != "nogather"),
                       do_compute=(rm != "gatheronly"))

    nc.compile()
    return nc


# ------------------------------------------------------------------- driver

_CACHE = {}


def _get_program_and_maps(x, edge_index, W0_1, W1_1, b1, W0_2, W1_2, b2):
    idx16, ldest, rp_a, rp_b, groups, nbtot = _pack(np.asarray(edge_index))

    x_pad = np.zeros((NP, F), dtype=np.float32)
    x_pad[:N] = np.asarray(x, dtype=np.float32)

    key = tuple(v for g in groups for v in g)
    if key not in _CACHE:
        _CACHE[key] = _build(groups, nbtot)
    nc = _CACHE[key]

    shared = {
        "w01": np.asarray(W0_1, np.float32),
        "w11": np.asarray(W1_1, np.float32),
        "b1": np.asarray(b1, np.float32).reshape(HID, 1),
        "w02": np.asarray(W0_2, np.float32),
        "w12": np.asarray(W1_2, np.float32),
        "b2": np.asarray(b2, np.float32).reshape(OUT, 1),
    }
    in_maps = []
    for c in range(C):
        m = dict(shared)
        m["x_st"] = np.ascontiguousarray(x_pad[c * SN:(c + 1) * SN])
        m["idx16"] = np.ascontiguousarray(idx16[c])
        m["ldst"] = np.ascontiguousarray(ldest[c])
        m["rpa"] = np.ascontiguousarray(rp_a[c])
        m["rpb"] = np.ascontiguousarray(rp_b[c])
        in_maps.append(m)
    return nc, in_maps


def kernel(x, edge_index, W0_1, W1_1, b1, W0_2, W1_2, b2, **kw):
    nc, in_maps = _get_program_and_maps(
        x, edge_index, W0_1, W1_1, b1, W0_2, W1_2, b2)

    from concourse.bass_utils import run_bass_kernel_spmd

    res = run_bass_kernel_spmd(nc, in_maps, core_ids=list(range(C)))
    outs = [res.results[c]["out"] for c in range(C)]
    full = np.concatenate(outs, axis=0)[:N]
    return full.astype(np.float32)



# revision 44
# speedup vs baseline: 84.6964x; 84.6964x over previous
"""ChebNet (K=2) GNN message passing on 8 TRN2 NeuronCores.

Strategy (edge sharding by destination stripe):
  - Sort edges by destination node; core c owns destinations
    [c*6272, (c+1)*6272) (N padded 50000 -> 50176 = 8*6272).
  - Host passes only integer index/offset data: per-edge source (col)
    indices, per-edge local destination slots, CSR row_ptr windows
    (full-graph and per-stripe). All floating point math (degree ->
    rsqrt, feature scaling, the two sparse matvecs, dense matmuls,
    log_softmax) runs on device.
  - L_hat matvec is factored: (L x)[i] = -dinv[i] * sum_e dinv[col_e] x[col_e]
    so the gather table is x' = dinv (.) x.
  - Layer 1 needs no collective: every core receives the FULL x (it is
    an ExternalInput) plus the full row_ptr, computes dinv for all
    50176 nodes and builds the bf16 gather table [NP, 128] locally
    (rows padded 64->128 bf16 elems = 256B, the dma_gather stride
    granularity).
  - Layer 2: h is computed distributed, so the compact bf16 stripe
    [SN, 64] is AllGathered (0.8MB/rank) and pad-expanded locally to
    the [NP, 128] table layout with one strided DMA.
  - Per 128-edge block a one-hot matrix S (iota == localdest, bf16) is
    built on DVE and PE computes G.T @ S into a PSUM window
    [64 feats, 128 dests] (bf16 operands = 1 cycle/row), accumulated
    over the window's blocks; the PSUM drain fuses the -dinv[dest]
    scale. Dense layer matmuls also run in bf16.
"""

import os
import sys

import numpy as np

sys.path.insert(0, "/opt/trn_rl_repo")

import concourse.bacc as bacc
import concourse.bass as bass
import concourse.tile as tile
from concourse import mybir
from concourse.masks import make_identity

FP32 = mybir.dt.float32
BF16 = mybir.dt.bfloat16
I32 = mybir.dt.int32
# One dma_gather per <=GMAX 128-edge blocks (multi-packet). Single-packet
# gathers above 512 indices wedge the device; multi-packet is stable at 1280.
GMAX = int(os.environ.get("CHEB_GMAX", "49"))
SINGLE_PACKET = os.environ.get("CHEB_SP", "0") == "1"

N = 50000
E = 800000
F = 64          # in dim
HID = 64
OUT = 40
C = 8           # cores
SN = 6272       # nodes per stripe (49 * 128)
NP = SN * C     # padded node count 50176
W = SN // 128   # 49 windows per core
NW = NP // 128  # 392 global windows
PAD = 128       # padded table row length (bf16 elems; 256B rows)
HALF = NP // 2  # 25088, int16-indexable table half
AX = mybir.AxisListType


# ---------------------------------------------------------------- host side


# layer-2 collective window-groups: dense1+h' for each group completes
# early, so its AllGather slice overlaps the rest of layer-1 compute.
WGRP = [(0, 12), (12, 24), (24, 36), (36, 49)]


def _r2(n):
    """Layer-2 table row for node n: group-major, then rank, then
    partition-major within the stripe - exactly the concatenation order
    the per-group AllGather slices produce."""
    c, m = n // SN, n % SN
    wl, p = m // 128, m % 128
    w0s = np.array([g[0] for g in WGRP], dtype=np.int64)
    g = np.searchsorted(w0s, wl, side="right") - 1
    w0 = w0s[g]
    ng = np.array([e - s for s, e in WGRP], dtype=np.int64)[g]
    base = w0 * 128 * C
    return base + c * ng * 128 + p * ng + (wl - w0)


def _group_edges(rs, rows):
    """Group the (dest-sorted) edges by (dest window, source-row parity)
    and pad each group to whole 128-edge blocks (counts maxed over cores
    so the SPMD program is uniform).

    rows = table row id of each edge's source; gather index is rows>>1
    (a 256B pair-row; fits int16 since NP/2 < 32768), the matmul slices
    the gathered 128-elem row at offset 64*(rows & 1).
    """
    win = rs >> 7
    par = (rows & 1).astype(np.int64)
    gid = win * 2 + par
    gorder = np.argsort(gid, kind="stable")
    rs = rs[gorder]
    rows = rows[gorder]
    gid = gid[gorder]

    gcnt = np.bincount(gid, minlength=C * W * 2).reshape(C, W, 2)
    nbw = (gcnt.max(axis=0) + 127) // 128          # [W, 2]
    # guarantee each window has at least one block so PSUM is initialized
    empty = nbw.sum(axis=1) == 0
    nbw[empty, 0] = 1
    nbtot = int(nbw.sum())

    # block base per (w, parity): even blocks then odd within a window
    wb = np.zeros(W * 2 + 1, dtype=np.int64)
    np.cumsum(nbw.reshape(-1), out=wb[1:])
    groups = [
        (int(wb[2 * w]), int(nbw[w, 0]), int(wb[2 * w + 1]), int(nbw[w, 1]))
        for w in range(W)
    ]

    flat_idx = np.zeros((C, nbtot * 128), dtype=np.int16)
    ldest = np.full((C, 128, nbtot), 255.0, dtype=np.float32)

    starts = np.zeros(C * W * 2 + 1, dtype=np.int64)
    np.cumsum(gcnt.reshape(-1), out=starts[1:])
    for c in range(C):
        for w in range(W):
            for h in range(2):
                g = (c * W + w) * 2 + h
                s, e = starts[g], starts[g + 1]
                m = e - s
                if m == 0:
                    continue
                base = wb[2 * w + h]
                ei = np.arange(m)
                b = base + (ei >> 7)
                p = ei & 127
                flat_idx[c, b * 128 + p] = (rows[s:e] >> 1).astype(np.int16)
                ldest[c, p, b] = (rs[s:e] - (c * SN + w * 128)).astype(
                    np.float32)

    # wrap: index i at [i % 16, i // 16], replicated down 8 partition groups
    S_tot = nbtot * 8
    idx16 = np.zeros((C, 128, S_tot), dtype=np.int16)
    wrapped = flat_idx.reshape(C, S_tot, 16).transpose(0, 2, 1)  # [C,16,S]
    for grp in range(8):
        idx16[:, grp * 16:(grp + 1) * 16, :] = wrapped

    return idx16, ldest, groups, nbtot


def _pack(edge_index: np.ndarray):
    """Integer-only preprocessing: sort/partition/pad the edge list.

    Two gather-index sets are emitted: layer 1 gathers from the locally
    built x' table in plain node order (row = node id); layer 2 gathers
    from the AllGather output, whose row order is _r2 (group-major).

    Returns:
      (idx16, ldest, groups, nbtot) per layer, per-core CSR row_ptr
      windows rp_a/rp_b [C, 128, W], full-graph row_ptr rp_af/rp_bf
      [128, NW] in the contiguous (p*NW + j) table-build layout.
    """
    row = np.asarray(edge_index[0], dtype=np.int64)
    col = np.asarray(edge_index[1], dtype=np.int64)

    cnt = np.bincount(row, minlength=NP)
    rp = np.zeros(NP + 1, dtype=np.int64)
    np.cumsum(cnt, out=rp[1:])

    order = np.argsort(row, kind="stable")
    rs = row[order]
    cs = col[order]
    keep = rs != cs
    rs = rs[keep]
    cs = cs[keep]

    pk1 = _group_edges(rs, cs)          # layer 1: row = node id
    pk2 = _group_edges(rs, _r2(cs))     # layer 2: row = r2(node)

    rpf = rp.astype(np.float64)
    assert rpf.max() < 2 ** 24
    idx = (np.arange(W)[None, :] * 128 + np.arange(128)[:, None])
    rp_a = np.zeros((C, 128, W), dtype=np.float32)
    rp_b = np.zeros((C, 128, W), dtype=np.float32)
    for c in range(C):
        base = c * SN
        rp_a[c] = rpf[base + idx].astype(np.float32)
        rp_b[c] = rpf[base + idx + 1].astype(np.float32)

    # full-graph row_ptr in the contiguous table-build layout:
    # SBUF (p, j) holds node p*NW + j
    idxf = np.arange(NP).reshape(128, NW)
    rp_af = rpf[idxf].astype(np.float32)
    rp_bf = rpf[idxf + 1].astype(np.float32)

    return pk1, pk2, rp_a, rp_b, rp_af, rp_bf


# -------------------------------------------------------------- bass program


def _build(groups1, nbtot1, groups2, nbtot2, replay=0):
    nc = bacc.Bacc(
        "TRN2",
        target_bir_lowering=False,
        debug=False,
        num_devices=C,
    )

    # --- I/O
    x_full = nc.dram_tensor("x_full", [NP, F], FP32, kind="ExternalInput").ap()
    x_st = nc.dram_tensor("x_st", [SN, F], FP32, kind="ExternalInput").ap()
    idx16a = nc.dram_tensor("idx16a", [128, nbtot1 * 8],
                            mybir.dt.int16, kind="ExternalInput").ap()
    ldsta = nc.dram_tensor("ldsta", [128, nbtot1], FP32,
                           kind="ExternalInput").ap()
    idx16b = nc.dram_tensor("idx16b", [128, nbtot2 * 8],
                            mybir.dt.int16, kind="ExternalInput").ap()
    ldstb = nc.dram_tensor("ldstb", [128, nbtot2], FP32,
                           kind="ExternalInput").ap()
    rpa = nc.dram_tensor("rpa", [128, W], FP32, kind="ExternalInput").ap()
    rpb = nc.dram_tensor("rpb", [128, W], FP32, kind="ExternalInput").ap()
    rpaf = nc.dram_tensor("rpaf", [128, NW], FP32, kind="ExternalInput").ap()
    rpbf = nc.dram_tensor("rpbf", [128, NW], FP32, kind="ExternalInput").ap()
    w01 = nc.dram_tensor("w01", [F, HID], FP32, kind="ExternalInput").ap()
    w11 = nc.dram_tensor("w11", [F, HID], FP32, kind="ExternalInput").ap()
    b1 = nc.dram_tensor("b1", [HID, 1], FP32, kind="ExternalInput").ap()
    w02 = nc.dram_tensor("w02", [HID, OUT], FP32, kind="ExternalInput").ap()
    w12 = nc.dram_tensor("w12", [HID, OUT], FP32, kind="ExternalInput").ap()
    b2 = nc.dram_tensor("b2", [OUT, 1], FP32, kind="ExternalInput").ap()
    out = nc.dram_tensor("out", [SN, OUT], FP32, kind="ExternalOutput").ap()

    # --- internal DRAM (compact bf16 gather tables + collective bounce)
    xp_c = nc.dram_tensor("xp_c", [NP, F], BF16).ap()
    hp_b = nc.dram_tensor("hp_b", [SN, F], BF16).ap()
    hp_full = nc.dram_tensor("hp_full", [NP, F], BF16,
                             addr_space="Shared").ap()
    dinv_d = nc.dram_tensor("dinv_d", [W, 128], FP32).ap()

    nwmax = max(g[1] + g[3] for g in groups1 + groups2)
    rg = [list(range(C))]

    with tile.TileContext(nc) as tc:
        with (
            tc.tile_pool(name="const", bufs=1) as cpool,
            tc.tile_pool(name="big", bufs=1) as bpool,
            tc.tile_pool(name="xload", bufs=2) as xpool,
            tc.tile_pool(name="work", bufs=2) as wpool,
            tc.tile_pool(name="gpool", bufs=3) as gpool,
            tc.tile_pool(name="spool", bufs=3) as spool,
            tc.tile_pool(name="psw", bufs=4, space="PSUM") as psw,
            tc.tile_pool(name="psd", bufs=2, space="PSUM") as psd,
            tc.tile_pool(name="pst", bufs=2, space="PSUM") as pst,
        ):
            def body():
                # ---- degree -> dinv, full graph [128, NW] (node = w*128+p)
                def make_dinv(rpa_ap, rpb_ap, nw, tagp):
                    rpa_s = cpool.tile([128, nw], FP32, tag=tagp + "a")
                    rpb_s = cpool.tile([128, nw], FP32, tag=tagp + "b")
                    nc.sync.dma_start(out=rpa_s[:], in_=rpa_ap)
                    nc.sync.dma_start(out=rpb_s[:], in_=rpb_ap)
                    deg = cpool.tile([128, nw], FP32, tag=tagp + "d")
                    nc.vector.tensor_tensor(out=deg[:], in0=rpb_s[:],
                                            in1=rpa_s[:],
                                            op=mybir.AluOpType.subtract)
                    degc = cpool.tile([128, nw], FP32, tag=tagp + "c")
                    nc.vector.tensor_scalar_max(degc[:], deg[:], 1.0)
                    rec = cpool.tile([128, nw], FP32, tag=tagp + "r")
                    nc.vector.reciprocal(rec[:], degc[:])
                    rsq = cpool.tile([128, nw], FP32, tag=tagp + "s")
                    nc.scalar.sqrt(rsq[:], rec[:])
                    msk = cpool.tile([128, nw], FP32, tag=tagp + "m")
                    nc.vector.tensor_scalar(out=msk[:], in0=deg[:],
                                            scalar1=0.5, scalar2=None,
                                            op0=mybir.AluOpType.is_ge)
                    dv = cpool.tile([128, nw], FP32, tag=tagp + "v")
                    nc.vector.tensor_tensor(out=dv[:], in0=rsq[:], in1=msk[:],
                                            op=mybir.AluOpType.mult)
                    return dv

                dinv_f = make_dinv(rpaf, rpbf, NW, "df")   # full graph
                dinv = make_dinv(rpa, rpb, W, "ds")        # own stripe

                ident = cpool.tile([128, 128], FP32, tag="ident")
                make_identity(nc, ident[:])
                ident_b = cpool.tile([128, 128], BF16, tag="ident_b")
                nc.vector.tensor_copy(ident_b[:], ident[:])

                # negative stripe dinv replicated over 64 partitions,
                # free-dim-major: transpose [128, W] -> [W, 128], negate,
                # bounce via DRAM with a partition-broadcast reload.
                ps_dt = pst.tile([W, 128], FP32, tag="pt")
                nc.tensor.transpose(out=ps_dt[:], in_=dinv[:],
                                    identity=ident[:])
                dt_s = cpool.tile([W, 128], FP32, tag="dt")
                nc.vector.tensor_scalar_mul(dt_s[:], ps_dt[:], -1.0)
                nc.sync.dma_start(out=dinv_d, in_=dt_s[:])
                ndrep = bpool.tile([F, SN], FP32, tag="ndrep")
                dflat = dinv_d.rearrange("w p -> (w p)")
                nc.sync.dma_start(
                    out=ndrep[:],
                    in_=bass.AP(dflat.tensor, dflat.offset,
                                [[0, F], [1, SN]]),
                )

                # ---- build bf16 gather table xp_c[n] = dinv[n]*x[n].
                # Fully contiguous: SBUF (p, j) holds node p*NW + j, so
                # both the x_full read and the table write are linear.
                XCH = 28            # nodes-per-partition per chunk
                x_lin = x_full.rearrange("(p j) f -> p (j f)", p=128)
                t_lin = xp_c.rearrange("(p j) f -> p (j f)", p=128)
                for ci in range(NW // XCH):
                    j0 = ci * XCH
                    xc = xpool.tile([128, XCH * F], FP32, tag="xc")
                    nc.sync.dma_start(
                        out=xc[:], in_=x_lin[:, j0 * F:(j0 + XCH) * F])
                    xs = xpool.tile([128, XCH * F], BF16, tag="xs")
                    dv_ap = dinv_f[:, j0:j0 + XCH]
                    nc.vector.tensor_tensor(
                        out=xs[:].rearrange("p (j f) -> p j f", j=XCH),
                        in0=xc[:].rearrange("p (j f) -> p j f", j=XCH),
                        in1=bass.AP(dv_ap.tensor, dv_ap.offset,
                                    [dv_ap.ap[0], dv_ap.ap[1], [0, F]]),
                        op=mybir.AluOpType.mult)
                    nc.sync.dma_start(
                        out=t_lin[:, j0 * F:(j0 + XCH) * F], in_=xs[:])

                # ---- remaining constants (queued after the table build
                # DMAs so the layer-1 gathers can start as early as possible)
                iota_i = cpool.tile([128, 128], I32, tag="iota_i")
                nc.gpsimd.iota(iota_i[:], pattern=[[1, 128]], base=0,
                               channel_multiplier=0)
                iota_f = cpool.tile([128, 128], BF16, tag="iota_f")
                nc.vector.tensor_copy(iota_f[:], iota_i[:])

                wts = {}
                for nm, src, shp in (("w01", w01, [F, HID]),
                                     ("w11", w11, [F, HID]),
                                     ("w02", w02, [HID, OUT]),
                                     ("w12", w12, [HID, OUT])):
                    f32 = cpool.tile(shp, FP32, tag=nm + "_f")
                    nc.sync.dma_start(out=f32[:], in_=src)
                    bft = cpool.tile(shp, BF16, tag=nm)
                    nc.vector.tensor_copy(bft[:], f32[:])
                    wts[nm] = bft
                b1_s = cpool.tile([HID, 1], FP32, tag="b1")
                b2_s = cpool.tile([OUT, 1], FP32, tag="b2")
                nc.sync.dma_start(out=b1_s[:], in_=b1)
                nc.sync.dma_start(out=b2_s[:], in_=b2)

                # one shared index/one-hot-source buffer pair, reloaded for
                # layer 2 (the WAR dep + reload hide under the collective)
                nbmx = max(nbtot1, nbtot2)

                def load_edges(idx_t, ldst_t, nbtot):
                    idx_s = cpool.tile([128, nbmx * 8], mybir.dt.int16,
                                       tag="ei")
                    nc.sync.dma_start(out=idx_s[:, :nbtot * 8], in_=idx_t)
                    ldst_f = cpool.tile([128, nbmx], FP32, tag="elf")
                    nc.sync.dma_start(out=ldst_f[:, :nbtot], in_=ldst_t)
                    ldst_s = cpool.tile([128, nbmx], BF16, tag="el")
                    nc.vector.tensor_copy(ldst_s[:, :nbtot],
                                          ldst_f[:, :nbtot])
                    return idx_s, ldst_s

                idx_a, ldst_a = load_edges(idx16a, ldsta, nbtot1)

                # ---- own stripe: xT (unscaled, bf16) for dense layer 1
                x_sb = bpool.tile([128, W * F], FP32, tag="x_sb")
                nc.sync.dma_start(
                    out=x_sb[:].rearrange("p (w f) -> p w f", w=W),
                    in_=x_st.rearrange("(w p) f -> p w f", p=128))
                xT = bpool.tile([F, SN], BF16, tag="xT")
                for w in range(W):
                    ps_t = pst.tile([F, 128], FP32, tag="pt")
                    nc.tensor.transpose(out=ps_t[:],
                                        in_=x_sb[:, w * F:(w + 1) * F],
                                        identity=ident[:])
                    nc.vector.tensor_copy(xT[:, w * 128:(w + 1) * 128],
                                          ps_t[:])

                # ---- sparse matvec: compact table [NP, F] bf16 gathered
                # as pair rows [NP/2, 128]; dst [F, SN] scaled by -dinv
                def matvec(table, dst, idx_s, ldst_s, groups, w0=0, w1=W):
                    tp = bass.AP(table.tensor, table.offset,
                                 [[PAD, HALF], [1, PAD]])
                    for w in range(w0, w1):
                        ev_b, ev_n, od_b, od_n = groups[w]
                        pw = psw.tile([F, 128], FP32, tag="pw", name="pw")
                        ntot = ev_n + od_n
                        s = spool.tile([128, nwmax * 128], BF16, tag="S",
                                       name="s")
                        io_ap = iota_f[:]
                        ld_ap = ldst_s[:, ev_b:ev_b + ntot]
                        nc.vector.tensor_tensor(
                            out=s[:, : ntot * 128].rearrange(
                                "p (b q) -> p b q", b=ntot),
                            in0=bass.AP(io_ap.tensor, io_ap.offset,
                                        [io_ap.ap[0], [0, ntot],
                                         io_ap.ap[1]]),
                            in1=bass.AP(ld_ap.tensor, ld_ap.offset,
                                        [ld_ap.ap[0], ld_ap.ap[1],
                                         [0, 128]]),
                            op=mybir.AluOpType.is_equal)
                        # one gather covers the whole window (even blocks
                        # then odd blocks are contiguous at ev_b); matmul
                        # slice offset per block is static from the counts
                        done = 0
                        for sub in range(0, ntot, GMAX):
                            base = ev_b + sub
                            n = min(GMAX, ntot - sub)
                            g = gpool.tile([128, min(nwmax, GMAX) * PAD],
                                           BF16, tag="G")
                            nc.gpsimd.dma_gather(
                                out_ap=g[:, : n * PAD].rearrange(
                                    "p (b f) -> p b f", b=n),
                                in_ap=tp,
                                idxs_ap=idx_s[:, base * 8:(base + n) * 8],
                                num_idxs=n * 128,
                                num_idxs_reg=n * 128,
                                elem_size=PAD,
                                single_packet=SINGLE_PACKET,
                            )
                            for b in range(n):
                                sb = sub + b
                                poff = 0 if sb < ev_n else F
                                nc.tensor.matmul(
                                    out=pw[:],
                                    lhsT=g[:, b * PAD + poff:
                                           b * PAD + poff + F],
                                    rhs=s[:, sb * 128:(sb + 1) * 128],
                                    start=(done == 0),
                                    stop=(done == ntot - 1))
                                done += 1
                        nc.vector.tensor_tensor(
                            out=dst[:, w * 128:(w + 1) * 128], in0=pw[:],
                            in1=ndrep[:, w * 128:(w + 1) * 128],
                            op=mybir.AluOpType.mult)

                txT = bpool.tile([F, SN], BF16, tag="txT")
                hT = bpool.tile([HID, SN], BF16, tag="hT")
                hp_sb = bpool.tile([128, W * F], BF16, tag="hp_sb")
                tx2T = bpool.tile([F, SN], BF16, tag="tx2T")
                out_sb = bpool.tile([128, W * OUT], FP32, tag="out_sb")
                nchunk = (SN + 511) // 512

                # ---- dense layer 1: hT = relu(W01.T @ xT + W11.T @ txT + b1)
                def dense1(c0, c1):
                    for lo in range(c0, c1, 512):
                        m = min(512, c1 - lo)
                        pd = psd.tile([HID, 512], FP32, tag="pd")
                        nc.tensor.matmul(out=pd[:, :m], lhsT=wts["w01"][:],
                                         rhs=xT[:, lo:lo + m], start=True,
                                         stop=False)
                        nc.tensor.matmul(out=pd[:, :m], lhsT=wts["w11"][:],
                                         rhs=txT[:, lo:lo + m], start=False,
                                         stop=True)
                        nc.scalar.activation(
                            out=hT[:, lo:lo + m], in_=pd[:, :m],
                            func=mybir.ActivationFunctionType.Relu,
                            bias=b1_s[:], scale=1.0)

                # ---- h' = dinv (.) h  (node-major, compact bf16)
                def hp_compute(w0, w1):
                    for w in range(w0, w1):
                        ps_h = pst.tile([128, F], BF16, tag="pt")
                        nc.tensor.transpose(out=ps_h[:],
                                            in_=hT[:, w * 128:(w + 1) * 128],
                                            identity=ident_b[:F, :F])
                        nc.vector.tensor_scalar_mul(
                            hp_sb[:, w * F:(w + 1) * F], ps_h[:],
                            dinv[:, w:w + 1])

                # ---- layer 1 + the h' AllGather, pipelined per window
                # group: each group's collective slice fires as soon as its
                # windows' dense1 + h' are done, overlapping the rest of
                # layer-1 compute. hp_b row = W0*128 + p*ng + (wl-W0), the
                # concat order the per-group AllGather produces (= _r2).
                for (gw0, gw1) in WGRP:
                    ng = gw1 - gw0
                    matvec(xp_c, txT, idx_a, ldst_a, groups1, gw0, gw1)
                    dense1(gw0 * 128, gw1 * 128)
                    hp_compute(gw0, gw1)
                    nc.sync.dma_start(
                        out=hp_b[gw0 * 128:gw1 * 128].rearrange(
                            "(p j) f -> p j f", p=128),
                        in_=hp_sb[:, gw0 * F:gw1 * F].rearrange(
                            "p (j f) -> p j f", j=ng))
                    nc.gpsimd.collective_compute(
                        "AllGather",
                        mybir.AluOpType.bypass,
                        ins=[hp_b[gw0 * 128:gw1 * 128]],
                        outs=[hp_full[gw0 * 128 * C:gw1 * 128 * C]],
                        replica_groups=rg,
                    )

                # ---- sparse matvec 2 (gathers straight from the AllGather
                # output - row order is _r2, no pad-expand needed)
                idx_b, ldst_b = load_edges(idx16b, ldstb, nbtot2)
                matvec(hp_full, tx2T, idx_b, ldst_b, groups2)

                # ---- dense layer 2 + bias + transpose + log_softmax
                def dense2_softmax():
                    o_all = bpool.tile([128, W * OUT], FP32, tag="o_all")
                    for i in range(nchunk):
                        lo = i * 512
                        m = min(512, SN - lo)
                        pd = psd.tile([OUT, 512], FP32, tag="pd")
                        nc.tensor.matmul(out=pd[:, :m], lhsT=wts["w02"][:],
                                         rhs=hT[:, lo:lo + m], start=True,
                                         stop=False)
                        nc.tensor.matmul(out=pd[:, :m], lhsT=wts["w12"][:],
                                         rhs=tx2T[:, lo:lo + m], start=False,
                                         stop=True)
                        ob = wpool.tile([OUT, 512], FP32, tag="ob")
                        nc.vector.tensor_scalar(
                            out=ob[:, :m], in0=pd[:, :m], scalar1=b2_s[:],
                            scalar2=None, op0=mybir.AluOpType.add)
                        for j in range(m // 128):
                            w = i * 4 + j
                            ps_o = pst.tile([128, OUT], FP32, tag="pt")
                            nc.tensor.transpose(
                                out=ps_o[:], in_=ob[:, j * 128:(j + 1) * 128],
                                identity=ident[:OUT, :OUT])
                            nc.vector.tensor_copy(
                                o_all[:, w * OUT:(w + 1) * OUT], ps_o[:])
                    e_all = bpool.tile([128, W * OUT], FP32, tag="e_all")
                    nc.scalar.activation(out=e_all[:], in_=o_all[:],
                                         func=mybir.ActivationFunctionType.Exp)
                    ssum = wpool.tile([128, W], FP32, tag="ssum")
                    nc.vector.tensor_reduce(
                        out=ssum[:],
                        in_=e_all[:].rearrange("p (w q) -> p w q", w=W),
                        axis=AX.X, op=mybir.AluOpType.add)
                    lns = wpool.tile([128, W], FP32, tag="lns")
                    nc.scalar.activation(out=lns[:], in_=ssum[:],
                                         func=mybir.ActivationFunctionType.Ln)
                    ln_ap = lns[:]
                    nc.vector.tensor_tensor(
                        out=out_sb[:].rearrange("p (w q) -> p w q", w=W),
                        in0=o_all[:].rearrange("p (w q) -> p w q", w=W),
                        in1=bass.AP(ln_ap.tensor, ln_ap.offset,
                                    [ln_ap.ap[0], ln_ap.ap[1], [0, OUT]]),
                        op=mybir.AluOpType.subtract)

                dense2_softmax()
                nc.sync.dma_start(
                    out=out.rearrange("(w p) f -> p w f", p=128),
                    in_=out_sb[:].rearrange("p (w f) -> p w f", w=W))

            for _ in range(replay + 1):
                body()

    nc.compile()
    return nc


# ------------------------------------------------------------------- driver

_CACHE = {}


def _get_program_and_maps(x, edge_index, W0_1, W1_1, b1, W0_2, W1_2, b2):
    pk1, pk2, rp_a, rp_b, rp_af, rp_bf = _pack(np.asarray(edge_index))
    idx16a, ldesta, groups1, nbtot1 = pk1
    idx16b, ldestb, groups2, nbtot2 = pk2

    x_pad = np.zeros((NP, F), dtype=np.float32)
    x_pad[:N] = np.asarray(x, dtype=np.float32)

    key = tuple(v for g in groups1 + groups2 for v in g)
    if key not in _CACHE:
        _CACHE[key] = _build(groups1, nbtot1, groups2, nbtot2)
    nc = _CACHE[key]

    shared = {
        "x_full": x_pad,
        "rpaf": rp_af,
        "rpbf": rp_bf,
        "w01": np.asarray(W0_1, np.float32),
        "w11": np.asarray(W1_1, np.float32),
        "b1": np.asarray(b1, np.float32).reshape(HID, 1),
        "w02": np.asarray(W0_2, np.float32),
        "w12": np.asarray(W1_2, np.float32),
        "b2": np.asarray(b2, np.float32).reshape(OUT, 1),
    }
    in_maps = []
    for c in range(C):
        m = dict(shared)
        m["x_st"] = np.ascontiguousarray(x_pad[c * SN:(c + 1) * SN])
        m["idx16a"] = np.ascontiguousarray(idx16a[c])
        m["ldsta"] = np.ascontiguousarray(ldesta[c])
        m["idx16b"] = np.ascontiguousarray(idx16b[c])
        m["ldstb"] = np.ascontiguousarray(ldestb[c])
        m["rpa"] = np.ascontiguousarray(rp_a[c])
        m["rpb"] = np.ascontiguousarray(rp_b[c])
        in_maps.append(m)
    return nc, in_maps


def kernel(x, edge_index, W0_1, W1_1, b1, W0_2, W1_2, b2, **kw):
    nc, in_maps = _get_program_and_maps(
        x, edge_index, W0_1, W1_1, b1, W0_2, W1_2, b2)

    from concourse.bass_utils import run_bass_kernel_spmd

    res = run_bass_kernel_spmd(nc, in_maps, core_ids=list(range(C)))
    outs = [res.results[c]["out"] for c in range(C)]
    full = np.concatenate(outs, axis=0)[:N]
    return full.astype(np.float32)
